# revision 13
# baseline (speedup 1.0000x reference)
"""Trainium2 Bass kernel for nn_LittleBitParallelLinear — fp8 DoubleRow version.

Computes y = ((x * h_in) @ sign(V)) * s @ sign(U).T * h_out.

Key idea: keep the TWO-stage factorized form (not the folded W) so that
both matmul weight operands are sign matrices (+-1), which are EXACT in
fp8e4m3.  Run every matmul in fp8 DoubleRow mode: one instruction
contracts a PAIR of 128-row k-subtiles at the same per-instruction cost
as a single bf16 matmul -> 2x MACs/instr.

fp8 quantization error lives only on the activations (x*h_in for stage
1, t*s for stage 2), ~2.65% rms per stage (3.74% end to end — too big).
So a partial hi/lo correction is added: for C1=24 of 32 IN-subtiles and
C2=12 of 16 RANK-subtiles, the fp8 residual (a - fp8(a)) is quantized
to a second fp8 operand and accumulated with the same +-1 weights.
Corrected subtiles pair up into extra DoubleRow instructions.  Measured
end-to-end rel err: ~1.86e-2 (< 2e-2 gate).

Instruction count per core: stage1 (16 hi + 12 lo pairs) x 16 rank-chunks
x 2 token-chunks = 896, stage2 (8 hi + 6 lo pairs) x 32 out-chunks x 2 =
896 -> 1792 DoubleRow matmuls vs the fp16 baseline's 2048: 0.875x PE
cycles.  Weight DMA halves (fp8): ~45MB/core total, fully overlapped.

x hi/lo split is host-side (dtype transform only); t's hi/lo is computed
on device: ACT copies PSUM*s -> a_hi (fp8), DVE computes
(PSUM*s - a_hi) -> a_lo (fp8) via scalar_tensor_tensor.  s folds into
the stage-1 drain scale; h_out folds into the stage-2 drain scale.

Token-parallel across 8 cores: core i handles tokens [i*1024,(i+1)*1024).
"""

import numpy as np
import ml_dtypes

P = 128
TOKENS, IN, OUT, RANK = 8192, 4096, 4096, 2048
N_CORES = 8
TOK = TOKENS // N_CORES   # 1024 tokens per core
FREE = 512                # moving free dim per matmul
NT = TOK // FREE          # 2 token chunks
NS1 = IN // P             # 32 stage-1 k-subtiles
NS2 = RANK // P           # 16 stage-2 k-subtiles
C1 = 24                   # corrected stage-1 subtiles
C2 = 12                   # corrected stage-2 subtiles
NP1 = NS1 // 2 + C1 // 2  # 28 stage-1 DoubleRow pairs
NP2 = NS2 // 2 + C2 // 2  # 14 stage-2 DoubleRow pairs
G = 2                     # out-chunks (of 128) per block / weight chunk
MB1 = RANK // P // G      # 8 stage-1 blocks
MB2 = OUT // P // G       # 16 stage-2 blocks

_cache = {}


def _build():
    import concourse.bacc as bacc
    import concourse.mybir as mybir
    import concourse.tile as tile

    f32 = mybir.dt.float32
    f16 = mybir.dt.float16
    f8 = mybir.dt.float8e4
    Copy = mybir.ActivationFunctionType.Copy
    DR = mybir.MatmulPerfMode.DoubleRow
    MUL = mybir.AluOpType.mult
    SUB = mybir.AluOpType.subtract

    nc = bacc.Bacc("TRN2", target_bir_lowering=False, debug=False)

    # Weight tensors hold only the 16/8 hi pairs: a lo-correction pair
    # p >= NH1 contracts subtiles (2(p-NH1), 2(p-NH1)+1) — exactly the
    # contents of hi pair p-NH1 — so lo matmuls reuse the hi SBUF slices.
    NH1 = NS1 // 2  # 16 stage-1 hi pairs
    NH2 = NS2 // 2  # 8 stage-2 hi pairs
    xs_ = nc.dram_tensor("xs", [P, NP1, 2, TOK], f8, kind="ExternalInput").ap()
    w1_ = nc.dram_tensor("w1", [MB1, P, NH1, 2, G * P], f8, kind="ExternalInput").ap()
    w2_ = nc.dram_tensor("w2", [MB2, P, NH2, 2, G * P], f8, kind="ExternalInput").ap()
    sv_ = nc.dram_tensor("sv", [P, NS2], f32, kind="ExternalInput").ap()
    hv_ = nc.dram_tensor("hv", [P, OUT // P], f32, kind="ExternalInput").ap()
    yT_ = nc.dram_tensor("yT", [OUT, TOK], f16, kind="ExternalOutput").ap()

    with tile.TileContext(nc) as tc:
        with (
            tc.tile_pool(name="x", bufs=1) as xpool,
            tc.tile_pool(name="w1", bufs=4) as w1pool,
            tc.tile_pool(name="w2", bufs=3) as w2pool,
            tc.tile_pool(name="a", bufs=1) as apool,
            tc.tile_pool(name="y", bufs=4) as ypool,
            tc.tile_pool(name="sc", bufs=1) as spool,
            tc.tile_pool(name="ps", bufs=8, space="PSUM") as psum,
        ):
            sv = spool.tile([P, NS2], f32)
            hv = spool.tile([P, OUT // P], f32)

            xs = xpool.tile([P, NP1, 2, TOK], f8)
            a8 = apool.tile([P, NP2, 2, TOK], f8)
            y3 = yT_.rearrange("(m p) t -> p m t", p=P)

            w1_tiles, w2_tiles = {}, {}

            def load_w1(mb):
                wt = w1pool.tile([P, NH1, 2, G * P], f8, name=f"w1_{mb}", tag="w1")
                nc.sync.dma_start(wt, w1_[mb])
                w1_tiles[mb] = wt

            def load_w2(ob):
                wt = w2pool.tile([P, NH2, 2, G * P], f8, name=f"w2_{ob}", tag="w2")
                nc.sync.dma_start(wt, w2_[ob])
                w2_tiles[ob] = wt

            # Head: interleave x and the first TWO weight chunks in fine
            # pair-chunks so the PE can start after the first ~0.6MB lands
            # (region-level tile deps).  The first two m-blocks are merged
            # into one 8-bank block below, so with DoubleRow at 0.5 cyc/col
            # the PE consumes a pair in ~1.4us vs ~1.3us of stream DMA —
            # the x stream stays just ahead instead of starving the PE.
            w1t0 = w1pool.tile([P, NH1, 2, G * P], f8, name="w1_0", tag="w1")
            w1t1 = w1pool.tile([P, NH1, 2, G * P], f8, name="w1_1", tag="w1")
            w1_tiles[0], w1_tiles[1] = w1t0, w1t1
            for pc0, pc1 in ((0, 1), (1, 3), (3, 7), (7, 14), (14, 21), (21, NP1)):
                nc.sync.dma_start(xs[:, pc0:pc1], xs_[:, pc0:pc1])
                if pc0 < NH1:
                    h1 = min(pc1, NH1)
                    nc.sync.dma_start(w1t0[:, pc0:h1], w1_[0][:, pc0:h1])
                    nc.sync.dma_start(w1t1[:, pc0:h1], w1_[1][:, pc0:h1])
                if pc0 == 1:
                    nc.sync.dma_start(sv, sv_)
                    nc.sync.dma_start(hv, hv_)
            load_w1(2)
            load_w1(3)
            next_w1 = 4
            next_w2 = 0

            def drain_s1(m, ps):
                for n in range(NT):
                    nsl = slice(n * FREE, (n + 1) * FREE)
                    ahi = a8[:, m // 2, m % 2, nsl]
                    nc.scalar.activation(ahi, ps[n], Copy, scale=sv[:, m : m + 1])
                    if m < C2:
                        alo = a8[:, NS2 // 2 + m // 2, m % 2, nsl]
                        nc.vector.scalar_tensor_tensor(
                            alo, ps[n], sv[:, m : m + 1], ahi, MUL, SUB
                        )

            # ---- stage 1: t = (x*h_in) @ sign(V), drained as a = fp8(t*s/2)
            # First block: m-chunks 0..3 merged (8 PSUM banks) for DMA runway.
            pss = {
                (g, n): psum.tile([P, FREE], f32, name=f"ps1A_{g}_{n}", tag="ps")
                for g in range(2 * G)
                for n in range(NT)
            }
            for p in range(NP1):
                wp = p if p < NH1 else p - NH1
                for g in range(2 * G):
                    wt = w1_tiles[g // G]
                    for n in range(NT):
                        nc.tensor.matmul(
                            pss[(g, n)],
                            lhsT=wt[:, wp, :, (g % G) * P : (g % G + 1) * P],
                            rhs=xs[:, p, :, n * FREE : (n + 1) * FREE],
                            start=(p == 0),
                            stop=(p == NP1 - 1),
                            perf_mode=DR,
                        )
            for g in range(2 * G):
                drain_s1(g, {n: pss[(g, n)] for n in range(NT)})
            w1_tiles.pop(0)
            w1_tiles.pop(1)

            for mb in range(2, MB1):
                if next_w1 <= min(mb + 2, MB1 - 1):
                    load_w1(next_w1)
                    next_w1 += 1
                if mb == MB1 - 1:
                    load_w2(0)
                    load_w2(1)
                    next_w2 = 2
                wt = w1_tiles[mb]
                pss = {
                    (g, n): psum.tile([P, FREE], f32, name=f"ps1_{mb}_{g}_{n}", tag="ps")
                    for g in range(G)
                    for n in range(NT)
                }
                for p in range(NP1):
                    wp = p if p < NH1 else p - NH1
                    for g in range(G):
                        for n in range(NT):
                            nc.tensor.matmul(
                                pss[(g, n)],
                                lhsT=wt[:, wp, :, g * P : (g + 1) * P],
                                rhs=xs[:, p, :, n * FREE : (n + 1) * FREE],
                                start=(p == 0),
                                stop=(p == NP1 - 1),
                                perf_mode=DR,
                            )
                for g in range(G):
                    drain_s1(mb * G + g, {n: pss[(g, n)] for n in range(NT)})
                w1_tiles.pop(mb)

            # ---- stage 2: y = (a @ sign(U).T) * h_out * 2
            for ob in range(MB2):
                if next_w2 <= min(ob + 2, MB2 - 1):
                    load_w2(next_w2)
                    next_w2 += 1
                wt = w2_tiles[ob]
                pss = {
                    (g, n): psum.tile([P, FREE], f32, name=f"ps2_{ob}_{g}_{n}", tag="ps")
                    for g in range(G)
                    for n in range(NT)
                }
                yt = ypool.tile([P, G, TOK], f16, name=f"yt_{ob}", tag="yt")
                last = ob == MB2 - 1
                if not last:
                    for q in range(NP2):
                        wq = q if q < NH2 else q - NH2
                        for g in range(G):
                            for n in range(NT):
                                nc.tensor.matmul(
                                    pss[(g, n)],
                                    lhsT=wt[:, wq, :, g * P : (g + 1) * P],
                                    rhs=a8[:, q, :, n * FREE : (n + 1) * FREE],
                                    start=(q == 0),
                                    stop=(q == NP2 - 1),
                                    perf_mode=DR,
                                )
                for g in range(G):
                    m2 = ob * G + g
                    if last:
                        # group-major (and n-major for the final group) so
                        # earlier groups drain and store while the PE is
                        # still on later ones — shortens the kernel tail.
                        for n in range(NT):
                            for q in range(NP2):
                                wq = q if q < NH2 else q - NH2
                                nc.tensor.matmul(
                                    pss[(g, n)],
                                    lhsT=wt[:, wq, :, g * P : (g + 1) * P],
                                    rhs=a8[:, q, :, n * FREE : (n + 1) * FREE],
                                    start=(q == 0),
                                    stop=(q == NP2 - 1),
                                    perf_mode=DR,
                                )
                            nsl = slice(n * FREE, (n + 1) * FREE)
                            nc.scalar.activation(
                                yt[:, g, nsl], pss[(g, n)], Copy,
                                scale=hv[:, m2 : m2 + 1],
                            )
                            nc.sync.dma_start(
                                y3[:, m2 : m2 + 1, nsl], yt[:, g : g + 1, nsl]
                            )
                    else:
                        for n in range(NT):
                            nsl = slice(n * FREE, (n + 1) * FREE)
                            nc.scalar.activation(
                                yt[:, g, nsl], pss[(g, n)], Copy,
                                scale=hv[:, m2 : m2 + 1],
                            )
                if not last:
                    nc.sync.dma_start(y3[:, ob * G : (ob + 1) * G], yt)
                w2_tiles.pop(ob)

    nc.compile()
    return nc


def _prep_weights(u, v, s, h_out):
    e4 = ml_dtypes.float8_e4m3fn
    bu = np.where(u >= 0, np.float32(1.0), np.float32(-1.0))
    bv = np.where(v >= 0, np.float32(1.0), np.float32(-1.0))

    # stage-1 weights, hi pairs only (lo pairs reuse them on device):
    # w1[mb, r, p, j, c] = bv[128*(2p+j)+r, mb*256+c]
    bv3 = bv.reshape(NS1, P, RANK)                  # [sub, r, rank]
    w1 = np.ascontiguousarray(
        bv3.reshape(NS1 // 2, 2, P, MB1, G * P).transpose(3, 2, 0, 1, 4)
    ).astype(e4)

    # stage-2 weights, hi pairs only:
    # w2[ob, r, q, j, c] = bu[ob*256+c, 128*(2q+j)+r]
    buT3 = np.ascontiguousarray(bu.T).reshape(NS2, P, OUT)
    w2 = np.ascontiguousarray(
        buT3.reshape(NS2 // 2, 2, P, MB2, G * P).transpose(3, 2, 0, 1, 4)
    ).astype(e4)

    sv = np.ascontiguousarray(s.reshape(NS2, P).T * np.float32(0.5))
    hv = np.ascontiguousarray(h_out.reshape(OUT // P, P).T * np.float32(2.0))
    return w1, w2, sv, hv


def _run(inputs, trace=False):
    from concourse.bass_utils import run_bass_kernel_spmd

    if "nc" not in _cache:
        _cache["nc"] = _build()
    nc = _cache["nc"]

    e4 = ml_dtypes.float8_e4m3fn
    x = np.asarray(inputs["x"], dtype=np.float32)
    u = np.asarray(inputs["u"], dtype=np.float32)
    v = np.asarray(inputs["v"], dtype=np.float32)
    s = np.asarray(inputs["s"], dtype=np.float32)
    h_in = np.asarray(inputs["h_in"], dtype=np.float32)
    h_out = np.asarray(inputs["h_out"], dtype=np.float32)

    w1, w2, sv, hv = _prep_weights(u, v, s, h_out)

    xh = x * h_in
    hi = xh.astype(e4)
    lo = (xh - hi.astype(np.float32)).astype(e4)

    in_maps = []
    for i in range(N_CORES):
        tsl = slice(i * TOK, (i + 1) * TOK)
        hiT = np.ascontiguousarray(hi[tsl].T).reshape(NS1, P, TOK)
        loT = np.ascontiguousarray(lo[tsl, : C1 * P].T).reshape(C1, P, TOK)
        xs8 = np.ascontiguousarray(
            np.concatenate([hiT, loT], axis=0)
            .reshape(NP1, 2, P, TOK)
            .transpose(2, 0, 1, 3)
        )
        in_maps.append({"xs": xs8, "w1": w1, "w2": w2, "sv": sv, "hv": hv})

    _cache["in_maps"] = in_maps
    res = run_bass_kernel_spmd(
        nc, in_maps, core_ids=list(range(N_CORES)), trace=trace
    )

    y = np.empty((TOKENS, OUT), dtype=np.float32)
    for i in range(N_CORES):
        y[i * TOK : (i + 1) * TOK, :] = res.results[i]["yT"].T.astype(np.float32)
    return y, res


def kernel(**inputs):
    y, _ = _run(inputs, trace=False)
    return y


# revision 14
# speedup vs baseline: 1.0283x; 1.0283x over previous
"""Trainium2 Bass kernel for nn_LittleBitParallelLinear — fp8 DoubleRow version.

Computes y = ((x * h_in) @ sign(V)) * s @ sign(U).T * h_out.

Key idea: keep the TWO-stage factorized form (not the folded W) so that
both matmul weight operands are sign matrices (+-1), which are EXACT in
fp8e4m3.  Run every matmul in fp8 DoubleRow mode: one instruction
contracts a PAIR of 128-row k-subtiles at the same per-instruction cost
as a single bf16 matmul -> 2x MACs/instr.

fp8 quantization error lives only on the activations (x*h_in for stage
1, t*s for stage 2), ~2.65% rms per stage (3.74% end to end — too big).
So a partial hi/lo correction is added: for C1=24 of 32 IN-subtiles and
C2=12 of 16 RANK-subtiles, the fp8 residual (a - fp8(a)) is quantized
to a second fp8 operand and accumulated with the same +-1 weights.
Corrected subtiles pair up into extra DoubleRow instructions.  Measured
end-to-end rel err: ~1.86e-2 (< 2e-2 gate).

Instruction count per core: stage1 (16 hi + 12 lo pairs) x 16 rank-chunks
x 2 token-chunks = 896, stage2 (8 hi + 6 lo pairs) x 32 out-chunks x 2 =
896 -> 1792 DoubleRow matmuls vs the fp16 baseline's 2048: 0.875x PE
cycles.  Weight DMA halves (fp8): ~45MB/core total, fully overlapped.

x hi/lo split is host-side (dtype transform only); t's hi/lo is computed
on device: ACT copies PSUM*s -> a_hi (fp8), DVE computes
(PSUM*s - a_hi) -> a_lo (fp8) via scalar_tensor_tensor.  s folds into
the stage-1 drain scale; h_out folds into the stage-2 drain scale.

Token-parallel across 8 cores: core i handles tokens [i*1024,(i+1)*1024).
"""

import numpy as np
import ml_dtypes

P = 128
TOKENS, IN, OUT, RANK = 8192, 4096, 4096, 2048
N_CORES = 8
TOK = TOKENS // N_CORES   # 1024 tokens per core
FREE = 512                # moving free dim per matmul
NT = TOK // FREE          # 2 token chunks
NS1 = IN // P             # 32 stage-1 k-subtiles
NS2 = RANK // P           # 16 stage-2 k-subtiles
C1 = 24                   # corrected stage-1 subtiles
C2 = 12                   # corrected stage-2 subtiles
NP1 = NS1 // 2 + C1 // 2  # 28 stage-1 DoubleRow pairs
NP2 = NS2 // 2 + C2 // 2  # 14 stage-2 DoubleRow pairs
G = 2                     # out-chunks (of 128) per block / weight chunk
MB1 = RANK // P // G      # 8 stage-1 blocks
MB2 = OUT // P // G       # 16 stage-2 blocks

_cache = {}


def _build():
    import concourse.bacc as bacc
    import concourse.mybir as mybir
    import concourse.tile as tile

    f32 = mybir.dt.float32
    f16 = mybir.dt.float16
    f8 = mybir.dt.float8e4
    Copy = mybir.ActivationFunctionType.Copy
    DR = mybir.MatmulPerfMode.DoubleRow
    MUL = mybir.AluOpType.mult
    SUB = mybir.AluOpType.subtract

    nc = bacc.Bacc("TRN2", target_bir_lowering=False, debug=False)

    # Weight tensors hold only the 16/8 hi pairs: a lo-correction pair
    # p >= NH1 contracts subtiles (2(p-NH1), 2(p-NH1)+1) — exactly the
    # contents of hi pair p-NH1 — so lo matmuls reuse the hi SBUF slices.
    NH1 = NS1 // 2  # 16 stage-1 hi pairs
    NH2 = NS2 // 2  # 8 stage-2 hi pairs
    xs_ = nc.dram_tensor("xs", [P, NP1, 2, TOK], f8, kind="ExternalInput").ap()
    w1_ = nc.dram_tensor("w1", [MB1, P, NH1, 2, G * P], f8, kind="ExternalInput").ap()
    w2_ = nc.dram_tensor("w2", [MB2, P, NH2, 2, G * P], f8, kind="ExternalInput").ap()
    sv_ = nc.dram_tensor("sv", [P, NS2], f32, kind="ExternalInput").ap()
    hv_ = nc.dram_tensor("hv", [P, OUT // P], f32, kind="ExternalInput").ap()
    yT_ = nc.dram_tensor("yT", [OUT, TOK], f16, kind="ExternalOutput").ap()

    with tile.TileContext(nc) as tc:
        with (
            tc.tile_pool(name="x", bufs=1) as xpool,
            tc.tile_pool(name="w1", bufs=4) as w1pool,
            tc.tile_pool(name="w2", bufs=3) as w2pool,
            tc.tile_pool(name="a", bufs=1) as apool,
            tc.tile_pool(name="y", bufs=4) as ypool,
            tc.tile_pool(name="sc", bufs=1) as spool,
            tc.tile_pool(name="ps", bufs=8, space="PSUM") as psum,
        ):
            sv = spool.tile([P, NS2], f32)
            hv = spool.tile([P, OUT // P], f32)

            xs = xpool.tile([P, NP1, 2, TOK], f8)
            a8 = apool.tile([P, NP2, 2, TOK], f8)
            y3 = yT_.rearrange("(m p) t -> p m t", p=P)

            w1_tiles, w2_tiles = {}, {}

            def load_w1(mb):
                wt = w1pool.tile([P, NH1, 2, G * P], f8, name=f"w1_{mb}", tag="w1")
                nc.sync.dma_start(wt, w1_[mb])
                w1_tiles[mb] = wt

            def load_w2(ob):
                wt = w2pool.tile([P, NH2, 2, G * P], f8, name=f"w2_{ob}", tag="w2")
                nc.sync.dma_start(wt, w2_[ob])
                w2_tiles[ob] = wt

            # Head: interleave x and the first TWO weight chunks in fine
            # pair-chunks so the PE can start after the first ~0.6MB lands
            # (region-level tile deps).  The first two m-blocks are merged
            # into one 8-bank block below, so with DoubleRow at 0.5 cyc/col
            # the PE consumes a pair in ~1.4us vs ~1.3us of stream DMA —
            # the x stream stays just ahead instead of starving the PE.
            w1t0 = w1pool.tile([P, NH1, 2, G * P], f8, name="w1_0", tag="w1")
            w1t1 = w1pool.tile([P, NH1, 2, G * P], f8, name="w1_1", tag="w1")
            w1_tiles[0], w1_tiles[1] = w1t0, w1t1
            for pc0, pc1 in ((0, 1), (1, 3), (3, 6), (6, 9), (9, 12), (12, 16), (16, 21), (21, NP1)):
                nc.sync.dma_start(xs[:, pc0:pc1], xs_[:, pc0:pc1])
                if pc0 < NH1:
                    h1 = min(pc1, NH1)
                    nc.sync.dma_start(w1t0[:, pc0:h1], w1_[0][:, pc0:h1])
                    nc.sync.dma_start(w1t1[:, pc0:h1], w1_[1][:, pc0:h1])
                if pc0 == 1:
                    nc.sync.dma_start(sv, sv_)
                    nc.sync.dma_start(hv, hv_)
            load_w1(2)
            load_w1(3)
            next_w1 = 4
            next_w2 = 0

            def drain_s1(m, ps):
                for n in range(NT):
                    nsl = slice(n * FREE, (n + 1) * FREE)
                    ahi = a8[:, m // 2, m % 2, nsl]
                    nc.scalar.activation(ahi, ps[n], Copy, scale=sv[:, m : m + 1])
                    if m < C2:
                        alo = a8[:, NS2 // 2 + m // 2, m % 2, nsl]
                        nc.vector.scalar_tensor_tensor(
                            alo, ps[n], sv[:, m : m + 1], ahi, MUL, SUB
                        )

            # ---- stage 1: t = (x*h_in) @ sign(V), drained as a = fp8(t*s/2)
            # First block: m-chunks 0..3 merged (8 PSUM banks) for DMA runway.
            pss = {
                (g, n): psum.tile([P, FREE], f32, name=f"ps1A_{g}_{n}", tag="ps")
                for g in range(2 * G)
                for n in range(NT)
            }
            for p in range(NP1):
                wp = p if p < NH1 else p - NH1
                for g in range(2 * G):
                    wt = w1_tiles[g // G]
                    for n in range(NT):
                        nc.tensor.matmul(
                            pss[(g, n)],
                            lhsT=wt[:, wp, :, (g % G) * P : (g % G + 1) * P],
                            rhs=xs[:, p, :, n * FREE : (n + 1) * FREE],
                            start=(p == 0),
                            stop=(p == NP1 - 1),
                            perf_mode=DR,
                        )
            for g in range(2 * G):
                drain_s1(g, {n: pss[(g, n)] for n in range(NT)})
            w1_tiles.pop(0)
            w1_tiles.pop(1)

            for mb in range(2, MB1):
                if next_w1 <= min(mb + 2, MB1 - 1):
                    load_w1(next_w1)
                    next_w1 += 1
                if mb == MB1 - 1:
                    load_w2(0)
                    load_w2(1)
                    next_w2 = 2
                wt = w1_tiles[mb]
                pss = {
                    (g, n): psum.tile([P, FREE], f32, name=f"ps1_{mb}_{g}_{n}", tag="ps")
                    for g in range(G)
                    for n in range(NT)
                }
                for p in range(NP1):
                    wp = p if p < NH1 else p - NH1
                    for g in range(G):
                        for n in range(NT):
                            nc.tensor.matmul(
                                pss[(g, n)],
                                lhsT=wt[:, wp, :, g * P : (g + 1) * P],
                                rhs=xs[:, p, :, n * FREE : (n + 1) * FREE],
                                start=(p == 0),
                                stop=(p == NP1 - 1),
                                perf_mode=DR,
                            )
                for g in range(G):
                    drain_s1(mb * G + g, {n: pss[(g, n)] for n in range(NT)})
                w1_tiles.pop(mb)

            # ---- stage 2: y = (a @ sign(U).T) * h_out * 2
            for ob in range(MB2):
                if next_w2 <= min(ob + 2, MB2 - 1):
                    load_w2(next_w2)
                    next_w2 += 1
                wt = w2_tiles[ob]
                pss = {
                    (g, n): psum.tile([P, FREE], f32, name=f"ps2_{ob}_{g}_{n}", tag="ps")
                    for g in range(G)
                    for n in range(NT)
                }
                yt = ypool.tile([P, G, TOK], f16, name=f"yt_{ob}", tag="yt")
                last = ob == MB2 - 1
                if not last:
                    for q in range(NP2):
                        wq = q if q < NH2 else q - NH2
                        for g in range(G):
                            for n in range(NT):
                                nc.tensor.matmul(
                                    pss[(g, n)],
                                    lhsT=wt[:, wq, :, g * P : (g + 1) * P],
                                    rhs=a8[:, q, :, n * FREE : (n + 1) * FREE],
                                    start=(q == 0),
                                    stop=(q == NP2 - 1),
                                    perf_mode=DR,
                                )
                for g in range(G):
                    m2 = ob * G + g
                    if last:
                        # group-major (and n-major for the final group) so
                        # earlier groups drain and store while the PE is
                        # still on later ones — shortens the kernel tail.
                        for n in range(NT):
                            for q in range(NP2):
                                wq = q if q < NH2 else q - NH2
                                nc.tensor.matmul(
                                    pss[(g, n)],
                                    lhsT=wt[:, wq, :, g * P : (g + 1) * P],
                                    rhs=a8[:, q, :, n * FREE : (n + 1) * FREE],
                                    start=(q == 0),
                                    stop=(q == NP2 - 1),
                                    perf_mode=DR,
                                )
                            nsl = slice(n * FREE, (n + 1) * FREE)
                            nc.scalar.activation(
                                yt[:, g, nsl], pss[(g, n)], Copy,
                                scale=hv[:, m2 : m2 + 1],
                            )
                            nc.sync.dma_start(
                                y3[:, m2 : m2 + 1, nsl], yt[:, g : g + 1, nsl]
                            )
                    else:
                        for n in range(NT):
                            nsl = slice(n * FREE, (n + 1) * FREE)
                            nc.scalar.activation(
                                yt[:, g, nsl], pss[(g, n)], Copy,
                                scale=hv[:, m2 : m2 + 1],
                            )
                if not last:
                    nc.sync.dma_start(y3[:, ob * G : (ob + 1) * G], yt)
                w2_tiles.pop(ob)

    nc.compile()
    return nc


def _prep_weights(u, v, s, h_out):
    e4 = ml_dtypes.float8_e4m3fn
    bu = np.where(u >= 0, np.float32(1.0), np.float32(-1.0))
    bv = np.where(v >= 0, np.float32(1.0), np.float32(-1.0))

    # stage-1 weights, hi pairs only (lo pairs reuse them on device):
    # w1[mb, r, p, j, c] = bv[128*(2p+j)+r, mb*256+c]
    bv3 = bv.reshape(NS1, P, RANK)                  # [sub, r, rank]
    w1 = np.ascontiguousarray(
        bv3.reshape(NS1 // 2, 2, P, MB1, G * P).transpose(3, 2, 0, 1, 4)
    ).astype(e4)

    # stage-2 weights, hi pairs only:
    # w2[ob, r, q, j, c] = bu[ob*256+c, 128*(2q+j)+r]
    buT3 = np.ascontiguousarray(bu.T).reshape(NS2, P, OUT)
    w2 = np.ascontiguousarray(
        buT3.reshape(NS2 // 2, 2, P, MB2, G * P).transpose(3, 2, 0, 1, 4)
    ).astype(e4)

    sv = np.ascontiguousarray(s.reshape(NS2, P).T * np.float32(0.5))
    hv = np.ascontiguousarray(h_out.reshape(OUT // P, P).T * np.float32(2.0))
    return w1, w2, sv, hv


def _run(inputs, trace=False):
    from concourse.bass_utils import run_bass_kernel_spmd

    if "nc" not in _cache:
        _cache["nc"] = _build()
    nc = _cache["nc"]

    e4 = ml_dtypes.float8_e4m3fn
    x = np.asarray(inputs["x"], dtype=np.float32)
    u = np.asarray(inputs["u"], dtype=np.float32)
    v = np.asarray(inputs["v"], dtype=np.float32)
    s = np.asarray(inputs["s"], dtype=np.float32)
    h_in = np.asarray(inputs["h_in"], dtype=np.float32)
    h_out = np.asarray(inputs["h_out"], dtype=np.float32)

    w1, w2, sv, hv = _prep_weights(u, v, s, h_out)

    xh = x * h_in
    hi = xh.astype(e4)
    lo = (xh - hi.astype(np.float32)).astype(e4)

    in_maps = []
    for i in range(N_CORES):
        tsl = slice(i * TOK, (i + 1) * TOK)
        hiT = np.ascontiguousarray(hi[tsl].T).reshape(NS1, P, TOK)
        loT = np.ascontiguousarray(lo[tsl, : C1 * P].T).reshape(C1, P, TOK)
        xs8 = np.ascontiguousarray(
            np.concatenate([hiT, loT], axis=0)
            .reshape(NP1, 2, P, TOK)
            .transpose(2, 0, 1, 3)
        )
        in_maps.append({"xs": xs8, "w1": w1, "w2": w2, "sv": sv, "hv": hv})

    _cache["in_maps"] = in_maps
    res = run_bass_kernel_spmd(
        nc, in_maps, core_ids=list(range(N_CORES)), trace=trace
    )

    y = np.empty((TOKENS, OUT), dtype=np.float32)
    for i in range(N_CORES):
        y[i * TOK : (i + 1) * TOK, :] = res.results[i]["yT"].T.astype(np.float32)
    return y, res


def kernel(**inputs):
    y, _ = _run(inputs, trace=False)
    return y


# revision 15
# speedup vs baseline: 1.1152x; 1.0845x over previous
"""Trainium2 Bass kernel for nn_LittleBitParallelLinear — fp8 DoubleRow version.

Computes y = ((x * h_in) @ sign(V)) * s @ sign(U).T * h_out.

Key idea: keep the TWO-stage factorized form (not the folded W) so that
both matmul weight operands are sign matrices (+-1), which are EXACT in
fp8e4m3.  Run every matmul in fp8 DoubleRow mode: one instruction
contracts a PAIR of 128-row k-subtiles at the same per-instruction cost
as a single bf16 matmul -> 2x MACs/instr.

fp8 quantization error lives only on the activations (x*h_in for stage
1, t*s for stage 2), ~2.65% rms per stage (3.74% end to end — too big).
So a partial hi/lo correction is added: for C1=24 of 32 IN-subtiles and
C2=12 of 16 RANK-subtiles, the fp8 residual (a - fp8(a)) is quantized
to a second fp8 operand and accumulated with the same +-1 weights.
Corrected subtiles pair up into extra DoubleRow instructions.  Measured
end-to-end rel err: ~1.86e-2 (< 2e-2 gate).

Instruction count per core: stage1 (16 hi + 12 lo pairs) x 16 rank-chunks
x 2 token-chunks = 896, stage2 (8 hi + 6 lo pairs) x 32 out-chunks x 2 =
896 -> 1792 DoubleRow matmuls vs the fp16 baseline's 2048: 0.875x PE
cycles.  Weight DMA halves (fp8): ~45MB/core total, fully overlapped.

x hi/lo split is host-side (dtype transform only); t's hi/lo is computed
on device: ACT copies PSUM*s -> a_hi (fp8), DVE computes
(PSUM*s - a_hi) -> a_lo (fp8) via scalar_tensor_tensor.  s folds into
the stage-1 drain scale; h_out folds into the stage-2 drain scale.

Token-parallel across 8 cores: core i handles tokens [i*1024,(i+1)*1024).
"""

import numpy as np
import ml_dtypes

P = 128
TOKENS, IN, OUT, RANK = 8192, 4096, 4096, 2048
N_CORES = 8
TOK = TOKENS // N_CORES   # 1024 tokens per core
FREE = 512                # moving free dim per matmul
NT = TOK // FREE          # 2 token chunks
NS1 = IN // P             # 32 stage-1 k-subtiles
NS2 = RANK // P           # 16 stage-2 k-subtiles
C1 = 24                   # corrected stage-1 subtiles
C2 = 12                   # corrected stage-2 subtiles
NP1 = NS1 // 2 + C1 // 2  # 28 stage-1 DoubleRow pairs
NP2 = NS2 // 2 + C2 // 2  # 14 stage-2 DoubleRow pairs
G = 2                     # out-chunks (of 128) per block / weight chunk
MB1 = RANK // P // G      # 8 stage-1 blocks
MB2 = OUT // P // G       # 16 stage-2 blocks

_cache = {}


def _build():
    import concourse.bacc as bacc
    import concourse.mybir as mybir
    import concourse.tile as tile

    f32 = mybir.dt.float32
    f16 = mybir.dt.float16
    f8 = mybir.dt.float8e4
    Copy = mybir.ActivationFunctionType.Copy
    DR = mybir.MatmulPerfMode.DoubleRow
    MUL = mybir.AluOpType.mult
    SUB = mybir.AluOpType.subtract

    nc = bacc.Bacc("TRN2", target_bir_lowering=False, debug=False)

    # Weight tensors hold only the 16/8 hi pairs: a lo-correction pair
    # p >= NH1 contracts subtiles (2(p-NH1), 2(p-NH1)+1) — exactly the
    # contents of hi pair p-NH1 — so lo matmuls reuse the hi SBUF slices.
    NH1 = NS1 // 2  # 16 stage-1 hi pairs
    NH2 = NS2 // 2  # 8 stage-2 hi pairs
    xs_ = nc.dram_tensor("xs", [P, NP1, 2, TOK], f8, kind="ExternalInput").ap()
    w1_ = nc.dram_tensor("w1", [MB1, P, NH1, 2, G * P], f8, kind="ExternalInput").ap()
    w2_ = nc.dram_tensor("w2", [MB2, P, NH2, 2, G * P], f8, kind="ExternalInput").ap()
    sv_ = nc.dram_tensor("sv", [P, NS2], f32, kind="ExternalInput").ap()
    hv_ = nc.dram_tensor("hv", [P, OUT // P], f32, kind="ExternalInput").ap()
    yT_ = nc.dram_tensor("yT", [OUT, TOK], f16, kind="ExternalOutput").ap()

    with tile.TileContext(nc) as tc:
        with (
            tc.tile_pool(name="x", bufs=1) as xpool,
            tc.tile_pool(name="w1", bufs=4) as w1pool,
            tc.tile_pool(name="w2", bufs=16) as w2pool,
            tc.tile_pool(name="a", bufs=1) as apool,
            tc.tile_pool(name="y", bufs=4) as ypool,
            tc.tile_pool(name="sc", bufs=1) as spool,
            tc.tile_pool(name="ps", bufs=8, space="PSUM") as psum,
        ):
            sv = spool.tile([P, NS2], f32)
            hv = spool.tile([P, OUT // P], f32)

            xs = xpool.tile([P, NP1, 2, TOK], f8)
            a8 = apool.tile([P, NP2, 2, TOK], f8)
            y3 = yT_.rearrange("(m p) t -> p m t", p=P)

            w1_tiles, w2_tiles = {}, {}

            def load_w1(mb):
                wt = w1pool.tile([P, NH1, 2, G * P], f8, name=f"w1_{mb}", tag="w1")
                nc.sync.dma_start(wt, w1_[mb])
                w1_tiles[mb] = wt

            def load_w2(ob):
                wt = w2pool.tile([P, NH2, 2, G * P], f8, name=f"w2_{ob}", tag="w2")
                nc.sync.dma_start(wt, w2_[ob])
                w2_tiles[ob] = wt

            # Head: interleave x and the first TWO weight chunks in fine
            # pair-chunks so the PE can start after the first ~0.6MB lands
            # (region-level tile deps).  The first two m-blocks are merged
            # into one 8-bank block below, so with DoubleRow at 0.5 cyc/col
            # the PE consumes a pair in ~1.4us vs ~1.3us of stream DMA —
            # the x stream stays just ahead instead of starving the PE.
            w1t0 = w1pool.tile([P, NH1, 2, G * P], f8, name="w1_0", tag="w1")
            w1t1 = w1pool.tile([P, NH1, 2, G * P], f8, name="w1_1", tag="w1")
            w1_tiles[0], w1_tiles[1] = w1t0, w1t1
            for pc0, pc1 in ((0, 1), (1, 3), (3, 6), (6, 9), (9, 12), (12, 16), (16, 21), (21, NP1)):
                nc.sync.dma_start(xs[:, pc0:pc1], xs_[:, pc0:pc1])
                if pc0 < NH1:
                    h1 = min(pc1, NH1)
                    nc.sync.dma_start(w1t0[:, pc0:h1], w1_[0][:, pc0:h1])
                    nc.sync.dma_start(w1t1[:, pc0:h1], w1_[1][:, pc0:h1])
                if pc0 == 1:
                    nc.sync.dma_start(sv, sv_)
                    nc.sync.dma_start(hv, hv_)
            load_w1(2)
            load_w1(3)
            next_w1 = 4
            next_w2 = 0

            def drain_s1(m, ps):
                for n in range(NT):
                    nsl = slice(n * FREE, (n + 1) * FREE)
                    ahi = a8[:, m // 2, m % 2, nsl]
                    nc.scalar.activation(ahi, ps[n], Copy, scale=sv[:, m : m + 1])
                    if m < C2:
                        alo = a8[:, NS2 // 2 + m // 2, m % 2, nsl]
                        nc.vector.scalar_tensor_tensor(
                            alo, ps[n], sv[:, m : m + 1], ahi, MUL, SUB
                        )

            # ---- stage 1: t = (x*h_in) @ sign(V), drained as a = fp8(t*s/2)
            # First block: m-chunks 0..3 merged (8 PSUM banks) for DMA runway.
            pss = {
                (g, n): psum.tile([P, FREE], f32, name=f"ps1A_{g}_{n}", tag="ps")
                for g in range(2 * G)
                for n in range(NT)
            }
            for p in range(NP1):
                wp = p if p < NH1 else p - NH1
                for g in range(2 * G):
                    wt = w1_tiles[g // G]
                    for n in range(NT):
                        nc.tensor.matmul(
                            pss[(g, n)],
                            lhsT=wt[:, wp, :, (g % G) * P : (g % G + 1) * P],
                            rhs=xs[:, p, :, n * FREE : (n + 1) * FREE],
                            start=(p == 0),
                            stop=(p == NP1 - 1),
                            perf_mode=DR,
                        )
            for g in range(2 * G):
                drain_s1(g, {n: pss[(g, n)] for n in range(NT)})
            w1_tiles.pop(0)
            w1_tiles.pop(1)

            for mb in range(2, MB1):
                if next_w1 <= min(mb + 2, MB1 - 1):
                    load_w1(next_w1)
                    next_w1 += 1
                while next_w2 < min(3 * (mb - 1), MB2):
                    load_w2(next_w2)
                    next_w2 += 1
                wt = w1_tiles[mb]
                pss = {
                    (g, n): psum.tile([P, FREE], f32, name=f"ps1_{mb}_{g}_{n}", tag="ps")
                    for g in range(G)
                    for n in range(NT)
                }
                for p in range(NP1):
                    wp = p if p < NH1 else p - NH1
                    for g in range(G):
                        for n in range(NT):
                            nc.tensor.matmul(
                                pss[(g, n)],
                                lhsT=wt[:, wp, :, g * P : (g + 1) * P],
                                rhs=xs[:, p, :, n * FREE : (n + 1) * FREE],
                                start=(p == 0),
                                stop=(p == NP1 - 1),
                                perf_mode=DR,
                            )
                for g in range(G):
                    drain_s1(mb * G + g, {n: pss[(g, n)] for n in range(NT)})
                w1_tiles.pop(mb)

            # ---- stage 2: y = (a @ sign(U).T) * h_out * 2
            for ob in range(MB2):
                while next_w2 < MB2:
                    load_w2(next_w2)
                    next_w2 += 1
                wt = w2_tiles[ob]
                pss = {
                    (g, n): psum.tile([P, FREE], f32, name=f"ps2_{ob}_{g}_{n}", tag="ps")
                    for g in range(G)
                    for n in range(NT)
                }
                yt = ypool.tile([P, G, TOK], f16, name=f"yt_{ob}", tag="yt")
                last = ob == MB2 - 1
                if not last:
                    for q in range(NP2):
                        wq = q if q < NH2 else q - NH2
                        for g in range(G):
                            for n in range(NT):
                                nc.tensor.matmul(
                                    pss[(g, n)],
                                    lhsT=wt[:, wq, :, g * P : (g + 1) * P],
                                    rhs=a8[:, q, :, n * FREE : (n + 1) * FREE],
                                    start=(q == 0),
                                    stop=(q == NP2 - 1),
                                    perf_mode=DR,
                                )
                for g in range(G):
                    m2 = ob * G + g
                    if last:
                        # group-major (and n-major for the final group) so
                        # earlier groups drain and store while the PE is
                        # still on later ones — shortens the kernel tail.
                        for n in range(NT):
                            for q in range(NP2):
                                wq = q if q < NH2 else q - NH2
                                nc.tensor.matmul(
                                    pss[(g, n)],
                                    lhsT=wt[:, wq, :, g * P : (g + 1) * P],
                                    rhs=a8[:, q, :, n * FREE : (n + 1) * FREE],
                                    start=(q == 0),
                                    stop=(q == NP2 - 1),
                                    perf_mode=DR,
                                )
                            nsl = slice(n * FREE, (n + 1) * FREE)
                            nc.scalar.activation(
                                yt[:, g, nsl], pss[(g, n)], Copy,
                                scale=hv[:, m2 : m2 + 1],
                            )
                            nc.sync.dma_start(
                                y3[:, m2 : m2 + 1, nsl], yt[:, g : g + 1, nsl]
                            )
                    else:
                        for n in range(NT):
                            nsl = slice(n * FREE, (n + 1) * FREE)
                            nc.scalar.activation(
                                yt[:, g, nsl], pss[(g, n)], Copy,
                                scale=hv[:, m2 : m2 + 1],
                            )
                if not last:
                    nc.sync.dma_start(y3[:, ob * G : (ob + 1) * G], yt)
                w2_tiles.pop(ob)

    nc.compile()
    return nc


def _prep_weights(u, v, s, h_out):
    e4 = ml_dtypes.float8_e4m3fn
    bu = np.where(u >= 0, np.float32(1.0), np.float32(-1.0))
    bv = np.where(v >= 0, np.float32(1.0), np.float32(-1.0))

    # stage-1 weights, hi pairs only (lo pairs reuse them on device):
    # w1[mb, r, p, j, c] = bv[128*(2p+j)+r, mb*256+c]
    bv3 = bv.reshape(NS1, P, RANK)                  # [sub, r, rank]
    w1 = np.ascontiguousarray(
        bv3.reshape(NS1 // 2, 2, P, MB1, G * P).transpose(3, 2, 0, 1, 4)
    ).astype(e4)

    # stage-2 weights, hi pairs only:
    # w2[ob, r, q, j, c] = bu[ob*256+c, 128*(2q+j)+r]
    buT3 = np.ascontiguousarray(bu.T).reshape(NS2, P, OUT)
    w2 = np.ascontiguousarray(
        buT3.reshape(NS2 // 2, 2, P, MB2, G * P).transpose(3, 2, 0, 1, 4)
    ).astype(e4)

    sv = np.ascontiguousarray(s.reshape(NS2, P).T * np.float32(0.5))
    hv = np.ascontiguousarray(h_out.reshape(OUT // P, P).T * np.float32(2.0))
    return w1, w2, sv, hv


def _run(inputs, trace=False):
    from concourse.bass_utils import run_bass_kernel_spmd

    if "nc" not in _cache:
        _cache["nc"] = _build()
    nc = _cache["nc"]

    e4 = ml_dtypes.float8_e4m3fn
    x = np.asarray(inputs["x"], dtype=np.float32)
    u = np.asarray(inputs["u"], dtype=np.float32)
    v = np.asarray(inputs["v"], dtype=np.float32)
    s = np.asarray(inputs["s"], dtype=np.float32)
    h_in = np.asarray(inputs["h_in"], dtype=np.float32)
    h_out = np.asarray(inputs["h_out"], dtype=np.float32)

    w1, w2, sv, hv = _prep_weights(u, v, s, h_out)

    xh = x * h_in
    hi = xh.astype(e4)
    lo = (xh - hi.astype(np.float32)).astype(e4)

    in_maps = []
    for i in range(N_CORES):
        tsl = slice(i * TOK, (i + 1) * TOK)
        hiT = np.ascontiguousarray(hi[tsl].T).reshape(NS1, P, TOK)
        loT = np.ascontiguousarray(lo[tsl, : C1 * P].T).reshape(C1, P, TOK)
        xs8 = np.ascontiguousarray(
            np.concatenate([hiT, loT], axis=0)
            .reshape(NP1, 2, P, TOK)
            .transpose(2, 0, 1, 3)
        )
        in_maps.append({"xs": xs8, "w1": w1, "w2": w2, "sv": sv, "hv": hv})

    _cache["in_maps"] = in_maps
    res = run_bass_kernel_spmd(
        nc, in_maps, core_ids=list(range(N_CORES)), trace=trace
    )

    y = np.empty((TOKENS, OUT), dtype=np.float32)
    for i in range(N_CORES):
        y[i * TOK : (i + 1) * TOK, :] = res.results[i]["yT"].T.astype(np.float32)
    return y, res


def kernel(**inputs):
    y, _ = _run(inputs, trace=False)
    return y
